# revision 2
# baseline (speedup 1.0000x reference)
"""ExLlama q4 dequant + matmul (tensor-parallel over out_features) on 8 trn2 cores.

Math (per core, N_loc = 28672/8 = 3584 columns):
  out[t,n] = sum_k x[t,k] * s[g(k),n] * (q[k,n] - (z[g(k),n]+1)) + bias[n]
           = sum_k x[t,k]*s[g,n]*q[k,n]  -  sum_g A[t,g]*(z+1)[g,n]*s[g,n] + bias[n]
  with A[t,g] = sum_{k in g} x[t,k] (host-computed, tiny).

Device pipeline per core ("Design W" — x stationary, dequantized weights moving):
  - qweight bytes are host-permuted into 16 "container tiles" [128, N_loc] uint16.
    Partition p of every tile holds k-values of group g = p//2 only, so ONE
    resident scale tile S_exp[p,n] = s[p//2, n] serves every tile.
    Container u16 at (jt, p, n) packs 4 nibbles: k = (p//2)*128 + (jt*2+p%2)*4 + c
    at bits 4c.
  - DVE extract (4x mode): (u16 & (0xF<<4c)) -> u16 = q * 16^c exact.
  - DVE scale (2x mode): tensor_tensor mult with S_exp -> w~ = q*s*16^c fp16.
  - PE: stationary = xt slice [128 k, 32 t] (x[t,k]*16^-c, host-permuted);
    moving = w~ in 7 chunks of N=512; PSUM [32 t, 3584 n] accumulates over
    all 64 (jt,c) passes.  One PSUM bank per 512-chunk, start on first pass.
  - Zero/bias fixup: one extra accumulating matmul per chunk with
    lhsT = r65 = [-A.T; 1] ([65, 32]), rhs = z65 = [(z+1)*s; bias] ([65, 3584],
    host-computed).
  - Single drain: ScalarE copy PSUM -> SBUF fp16, DMA out [32, 3584].
"""

import numpy as np

GROUP_SIZE = 128
IN_FEATURES = 8192
OUT_FEATURES = 28672
TOKENS = 32
N_CORES = 8
N_LOC = OUT_FEATURES // N_CORES          # 3584
NJT = IN_FEATURES // (GROUP_SIZE * 4)    # 16 container tiles
G = IN_FEATURES // GROUP_SIZE            # 64 groups
MASKS = (0x000F, 0x00F0, 0x0F00, 0xF000)
MMCH = 512                               # moving cols per matmul (1 PSUM bank)

_PROGRAM_CACHE = {}


# ---------------------------------------------------------------- host prep

def _k_index_map():
    """k(jt, p, c) = (p//2)*128 + (jt*2 + p%2)*4 + c  -> [NJT, 128, 4] int."""
    jt = np.arange(NJT)[:, None, None]
    p = np.arange(128)[None, :, None]
    c = np.arange(4)[None, None, :]
    return (p // 2) * GROUP_SIZE + (jt * 2 + (p % 2)) * 4 + c


def _prep_wq(qw_slice):
    """[1024, N_loc] int32 -> [NJT, 128, N_loc] uint16 container tiles."""
    nloc = qw_slice.shape[1]
    qb = np.ascontiguousarray(qw_slice).view(np.uint8).reshape(1024, nloc, 4)
    # byte kp = 4*kk + b holds nibbles for k = 2kp (lo), 2kp+1 (hi)
    qb_kp = np.ascontiguousarray(qb.transpose(0, 2, 1)).reshape(4096, nloc)
    jt = np.arange(NJT)[:, None]
    p = np.arange(128)[None, :]
    kp0 = (p // 2) * 64 + (jt * 2 + (p % 2)) * 2      # [NJT, 128]
    b2 = np.stack([qb_kp[kp0], qb_kp[kp0 + 1]], axis=-1)  # [NJT,128,nloc,2] u8
    return np.ascontiguousarray(b2).view(np.uint16)[..., 0]


def _prep_xt(x):
    """x [32, 8192] fp16 -> xt [128, NJT*4*32] fp16, tile (jt,c) at cols (jt*4+c)*32."""
    kmap = _k_index_map()                              # [NJT, 128, 4]
    xf = x.astype(np.float32)
    xt = np.empty((128, NJT * 4 * TOKENS), dtype=np.float16)
    for jt in range(NJT):
        for c in range(4):
            blk = xf[:, kmap[jt, :, c]].T * (2.0 ** (-4 * c))   # [128, 32]
            xt[:, (jt * 4 + c) * TOKENS:(jt * 4 + c + 1) * TOKENS] = \
                blk.astype(np.float16)
    return xt


def _prep_r65(x):
    """[-A.T ; ones] -> [65, 32] fp16, A[t,g] = sum_{k in g} x[t,k] (fp32)."""
    A = x.astype(np.float32).reshape(TOKENS, G, GROUP_SIZE).sum(axis=2)  # [32, 64]
    r = np.empty((G + 1, TOKENS), dtype=np.float16)
    r[:G] = (-A.T).astype(np.float16)
    r[G] = 1.0
    return r


def _prep_z65(qz_slice, s_slice, b_slice):
    """z65 [65, nloc] fp16: rows 0..63 = (z+1)*s, row 64 = bias."""
    nloc = s_slice.shape[1]
    shifts = (np.arange(8, dtype=np.uint32) * 4)[None, None, :]
    z = ((qz_slice.astype(np.uint32)[:, :, None] >> shifts) & 15)
    z = z.reshape(G, nloc).astype(np.float32)
    z65 = np.empty((G + 1, nloc), dtype=np.float16)
    z65[:G] = ((z + 1.0) * s_slice.astype(np.float32)).astype(np.float16)
    z65[G] = b_slice
    return z65


# ---------------------------------------------------------------- device program

def _build_program(nloc, loop_r=1):
    import concourse.bacc as bacc
    import concourse.mybir as mybir
    import concourse.tile as tile
    from concourse.alu_op_type import AluOpType

    dt = mybir.dt
    nch = nloc // MMCH

    nc = bacc.Bacc("TRN2", target_bir_lowering=False, debug=False,
                   num_devices=N_CORES)

    wq_d = nc.dram_tensor("wq", [NJT, 128, nloc], dt.uint16, kind="ExternalInput")
    sexp_d = nc.dram_tensor("sexp", [128, nloc], dt.float16, kind="ExternalInput")
    z65_d = nc.dram_tensor("z65", [G + 1, nloc], dt.float16, kind="ExternalInput")
    xt_d = nc.dram_tensor("xt", [128, NJT * 4 * TOKENS], dt.float16,
                          kind="ExternalInput")
    r65_d = nc.dram_tensor("r65", [G + 1, TOKENS], dt.float16, kind="ExternalInput")
    out_d = nc.dram_tensor("out", [TOKENS, nloc], dt.float16,
                           kind="ExternalOutput")

    with tile.TileContext(nc) as tc:
        with (
            tc.tile_pool(name="const", bufs=1) as const_pool,
            tc.tile_pool(name="wq", bufs=3) as wq_pool,
            tc.tile_pool(name="ext", bufs=4) as ext_pool,
            tc.tile_pool(name="sw", bufs=6) as sw_pool,
            tc.tile_pool(name="psum", bufs=1, space="PSUM") as psum_pool,
        ):
            def emit_body():
                sexp = const_pool.tile([128, nloc], dt.float16, tag="sexp")
                nc.sync.dma_start(sexp[:], sexp_d[:])
                xt = const_pool.tile([128, NJT * 4 * TOKENS], dt.float16,
                                     tag="xt")
                nc.sync.dma_start(xt[:], xt_d[:])
                z65 = const_pool.tile([G + 1, nloc], dt.float16, tag="z65")
                nc.sync.dma_start(z65[:], z65_d[:])
                r65 = const_pool.tile([G + 1, TOKENS], dt.float16, tag="r65")
                nc.sync.dma_start(r65[:], r65_d[:])

                psum = psum_pool.tile([TOKENS, nch * MMCH], dt.float32,
                                      tag="acc")

                for jt in range(NJT):
                    wq_t = wq_pool.tile([128, nloc], dt.uint16)
                    nc.sync.dma_start(wq_t[:], wq_d[jt, :, :])
                    for c in range(4):
                        ext = ext_pool.tile([128, nloc], dt.uint16)
                        nc.vector.tensor_scalar(
                            ext[:], wq_t[:], MASKS[c], None,
                            AluOpType.bitwise_and)
                        sw = sw_pool.tile([128, nloc], dt.float16)
                        nc.vector.tensor_tensor(
                            sw[:], ext[:], sexp[:], AluOpType.mult)
                        xcol = (jt * 4 + c) * TOKENS
                        for ci in range(nch):
                            nc.tensor.matmul(
                                psum[:, ci * MMCH:(ci + 1) * MMCH],
                                xt[:, xcol:xcol + TOKENS],
                                sw[:, ci * MMCH:(ci + 1) * MMCH],
                                start=(jt == 0 and c == 0),
                                stop=False)

                for ci in range(nch):
                    nc.tensor.matmul(
                        psum[:, ci * MMCH:(ci + 1) * MMCH],
                        r65[:],
                        z65[:, ci * MMCH:(ci + 1) * MMCH],
                        start=False,
                        stop=True)

                stg = const_pool.tile([TOKENS, nch * MMCH], dt.float16,
                                      tag="stg")
                nc.scalar.copy(stg[:], psum[:])
                nc.sync.dma_start(out_d[:], stg[:])

            if loop_r == 1:
                emit_body()
            else:
                with tc.For_i(0, loop_r, 1):
                    emit_body()

    nc.compile()
    return nc


def _get_program(nloc=N_LOC):
    if nloc not in _PROGRAM_CACHE:
        _PROGRAM_CACHE[nloc] = _build_program(nloc)
    return _PROGRAM_CACHE[nloc]


# ---------------------------------------------------------------- entry point

def make_in_maps(x, qweight, qzeros, scales, bias, nloc=N_LOC, n_cores=N_CORES):
    x = np.asarray(x)
    qweight = np.asarray(qweight)
    qzeros = np.asarray(qzeros)
    scales = np.asarray(scales)
    bias = np.asarray(bias)

    xt = _prep_xt(x)
    r65 = _prep_r65(x)
    in_maps = []
    for core in range(n_cores):
        n0, n1 = core * nloc, (core + 1) * nloc
        s_slice = np.ascontiguousarray(scales[:, n0:n1]).astype(np.float16)
        qz_slice = np.ascontiguousarray(qzeros[:, n0 // 8:n1 // 8]).view(
            np.uint32)
        b_slice = np.ascontiguousarray(bias[n0:n1]).astype(np.float16)
        in_maps.append({
            "wq": _prep_wq(qweight[:, n0:n1]),
            "sexp": np.repeat(s_slice, 2, axis=0),
            "z65": _prep_z65(qz_slice, s_slice, b_slice),
            "xt": xt,
            "r65": r65,
        })
    return in_maps


def assemble_output(results, nloc=N_LOC, n_cores=N_CORES):
    parts = [np.asarray(results[core]["out"]) for core in range(n_cores)]
    return np.ascontiguousarray(np.concatenate(parts, axis=1))


def kernel(x, qweight, qzeros, scales, bias):
    from concourse.bass_utils import run_bass_kernel_spmd

    nc = _get_program()
    in_maps = make_in_maps(x, qweight, qzeros, scales, bias)
    res = run_bass_kernel_spmd(nc, in_maps, list(range(N_CORES)))
    return assemble_output(res.results)
